# revision 24
# baseline (speedup 1.0000x reference)
"""Distributed multi-head attention kernel for 8 TRN2 NeuronCores (v4).

Reference problem (hardcoded):
    hidden_states [1, 1024, 1, 2048] f32, Wq/Wk/Wv [1024, 1024],
    Wo [1024, 1024], bo [1024].  16 heads x 64 dim, seq 2048.

Sharding: tensor-parallel over heads.  Core i computes heads (2i, 2i+1):
QKV projections for its 128 channels, per-head scores, exp, PV, normalize,
AllGather of the attn block, then a row shard of the output projection
(+bias).  Host concatenates row shards.

v4 — all matmuls K=128 via BLOCK-DIAGONAL head packing (HW calibration
showed K=64 matmuls run at ~2x cost/col and single-shot K=64 at ~443 ns vs
~246 ns for K=128):
  - scores: stationary kbd[.., blk, ..] is [128, 128] block-diagonal
    (k_h0 in the top-left 64x64, k_h1 bottom-right); moving q_sb [128, n]
    holds both heads' d stacked.  Output tile row p<64 = S_h0[kpos, q],
    p>=64 = S_h1[kpos, q] for the same 64 k-positions `blk`.
  - exp: DVE copies score tiles PSUM->SBUF bf16 (~0.5 ns/col; the v2
    kernel's ACT-exp-from-PSUM ran at ~5.7 ns/col and dominated), ACT
    exps bf16->bf16 from SBUF (~0.2 ns/col).
  - PV: stationary vbd[.., blk, ..] is [128, 128] block-diagonal vT built
    from two partition-offset transposes per 64-block; output rows are the
    128 attn channels directly.
  - softmax denominators: ones-pair stationary [128, 2] summing rows 0:64
    (h0) and 64:128 (h1) of each exp tile into a [2, 512] accumulator.
  - x DMA in [128, 1024] chunks (2 KB lines; 1 KB lines ran at ~half BW).
  - post prefetches the gathered-attn row blocks 4 deep.
bench mode (loop_r=N) wraps pre+post in a single hardware For_i.
"""

import numpy as np
import ml_dtypes

import concourse.bass as bass
import concourse.mybir as mybir
import concourse.tile as tile
from concourse import bacc
from concourse.bass import ts, ds
from concourse.bass_utils import run_bass_kernel_spmd

S = 2048          # sequence length
C = 1024          # query dim == inner dim
P = 128           # partitions / per-core channel count
D = 64            # head dim
HC = 2            # heads per core
N_CORES = 8
KC = C // P       # 8 contraction chunks for the projections
NBLK = S // D     # 32 key-position blocks of 64
NB = S // 512     # 4 free-dim blocks of 512
FP32 = mybir.dt.float32
BF16 = mybir.dt.bfloat16
AFT = mybir.ActivationFunctionType


def build(loop_r=None, reps=1, part="full"):
    nc = bacc.Bacc("TRN2", target_bir_lowering=False, debug=False,
                   num_devices=N_CORES)
    x_d = nc.dram_tensor("x", [C, S], BF16, kind="ExternalInput")
    wq_d = nc.dram_tensor("wqT", [C, P], BF16, kind="ExternalInput")
    wk_d = nc.dram_tensor("wkT", [C, P], BF16, kind="ExternalInput")
    wv_d = nc.dram_tensor("wvT", [C, P], BF16, kind="ExternalInput")
    wo_d = nc.dram_tensor("woT", [C, P], BF16, kind="ExternalInput")
    bo_d = nc.dram_tensor("bo", [P, 1], FP32, kind="ExternalInput")
    id_d = nc.dram_tensor("ident", [P, P], BF16, kind="ExternalInput")
    ones_d = nc.dram_tensor("ones", [P, 1], BF16, kind="ExternalInput")
    dnw_d = nc.dram_tensor("dnw", [P, 2], BF16, kind="ExternalInput")
    dnsel_d = nc.dram_tensor("dnsel", [2, P], BF16, kind="ExternalInput")
    out_d = nc.dram_tensor("out", [P, S], FP32, kind="ExternalOutput")

    with tile.TileContext(nc) as tc:
        with (
            tc.tile_pool(name="const", bufs=1) as cpool,
            tc.tile_pool(name="big", bufs=1) as big,
            tc.tile_pool(name="sc", bufs=3) as scpool,
            tc.tile_pool(name="rhs", bufs=4) as rpool,
            tc.tile_pool(name="small", bufs=4) as spool,
            tc.tile_pool(name="psum", bufs=8, space="PSUM") as ppool,
            tc.tile_pool(name="dram", bufs=1, space="DRAM") as dpool,
        ):
            # ---- constants / weights (outside any bench loop) ----
            ident = cpool.tile([P, P], BF16, tag="ident")
            nc.sync.dma_start(ident[:], id_d.ap())
            ones_sb = cpool.tile([P, 1], BF16, tag="ones")
            nc.sync.dma_start(ones_sb[:], ones_d.ap())
            w_sb = {}
            for name, dram in (("q", wq_d), ("k", wk_d), ("v", wv_d),
                               ("o", wo_d)):
                t = cpool.tile([P, KC, P], BF16, tag=f"w{name}")
                nc.sync.dma_start(
                    t[:], dram.ap().rearrange("(kc p) m -> p kc m", p=P))
                w_sb[name] = t
            bo_sb = cpool.tile([P, 1], FP32, tag="bo")
            nc.sync.dma_start(bo_sb[:], bo_d.ap())
            # absorb the exp table load into the DMA lead-in
            warm = cpool.tile([P, 1], FP32, tag="warm")
            nc.scalar.activation(warm[:], bo_sb[:], AFT.Exp)
            # ones-pair stationary for the denominator matmuls:
            # col 0 sums rows 0:64 (h0), col 1 sums rows 64:128 (h1)
            dn_w = cpool.tile([P, 2], BF16, tag="dnw")
            nc.sync.dma_start(dn_w[:], dnw_d.ap())
            # selector for the PE denominator broadcast: col c takes rec row
            # 0 (c < 64) or row 1 (c >= 64)
            dn_sel = cpool.tile([2, P], BF16, tag="dnsel")
            nc.sync.dma_start(dn_sel[:], dnsel_d.ap())
            # block-diagonal stationaries; the zero halves are written once
            kbd = big.tile([P, NBLK, P], BF16, tag="kbd")
            vbd = big.tile([P, NBLK, P], BF16, tag="vbd")
            nc.vector.memset(kbd[:], 0.0)
            nc.vector.memset(vbd[:], 0.0)

            x_sb = big.tile([P, KC, S], BF16, tag="x")
            proj = {}
            for name in ("k", "q", "v"):
                proj[name] = big.tile([P, S], BF16, tag=f"{name}sb",
                                      name=f"{name}sb")
            e_sb = big.tile([P, NBLK, S // 2], BF16, tag="esb")
            attn_sb = big.tile([P, S], BF16, tag="attn")
            out_sb = big.tile([P, S], FP32, tag="outsb")
            ag_in = [dpool.tile([P, S // 2], BF16, tag=f"agin{i}",
                                name=f"agin{i}") for i in range(2)]
            ag_out = [dpool.tile([C, S // 2], BF16, tag=f"agout{i}",
                                 addr_space="Shared", name=f"agout{i}")
                      for i in range(2)]

            def emit_xdma():
                # x into SBUF in [128, 1024] chunks (2 KB DMA lines)
                x_view = x_d.ap().rearrange("(kc p) s -> kc p s", kc=KC)
                for half in range(2):
                    for kc in range(KC):
                        nc.sync.dma_start(
                            x_sb[:, kc, ds(half * 1024, 1024)],
                            x_view[kc][:, ds(half * 1024, 1024)])

            def emit_proj(name, nbs):
                for nb in nbs:
                    ps = ppool.tile([P, 512], FP32, tag="ps",
                                    name=f"{name}{nb}_ps")
                    for kc in range(KC):
                        nc.tensor.matmul(
                            ps[:], w_sb[name][:, kc, :],
                            x_sb[:, kc, ts(nb, 512)],
                            start=(kc == 0), stop=(kc == KC - 1))
                    nc.vector.tensor_copy(proj[name][:, ts(nb, 512)], ps[:])

            def emit_kbd():
                k_sb = proj["k"]
                for blk in range(NBLK):
                    nc.vector.tensor_copy(kbd[0:D, blk, 0:D],
                                          k_sb[0:D, ts(blk, D)])
                    nc.vector.tensor_copy(kbd[D:P, blk, D:P],
                                          k_sb[D:P, ts(blk, D)])

            def emit_vbd():
                # two partition-offset transposes per 64-block, then
                # partition-aligned copies into the block-diagonal layout
                v_sb = proj["v"]
                for blk in range(NBLK):
                    tp = ppool.tile([P, P], BF16, tag="ps", name=f"tp{blk}")
                    nc.tensor.transpose(tp[0:D, :], v_sb[:, ts(blk, D)],
                                        ident[:])
                    nc.tensor.transpose(tp[D:P, :], v_sb[:, ts(blk, D)],
                                        ident[:])
                    nc.vector.tensor_copy(vbd[0:D, blk, 0:D],
                                          tp[0:D, 0:D])
                    nc.vector.tensor_copy(vbd[D:P, blk, D:P],
                                          tp[D:P, D:P])

            def emit_scores(qh):
                # per blk: [128,128] block-diag stationary, 2 single-shot
                # matmuls; DVE copies PSUM->SBUF bf16; ACT exps into e_sb
                q_sb = proj["q"]
                for blk in range(NBLK):
                    sts = []
                    for j in range(2):
                        st = ppool.tile([P, 512], FP32, tag="ps",
                                        name=f"st{blk}_{j}")
                        nc.tensor.matmul(
                            st[:], kbd[:, blk, :],
                            q_sb[:, ds(qh * 1024 + j * 512, 512)],
                            start=True, stop=True)
                        sts.append(st)
                    sc = scpool.tile([P, S // 2], BF16, tag="sc",
                                     name=f"sc{blk}")
                    for j in range(2):
                        nc.vector.tensor_copy(sc[:, ts(j, 512)], sts[j][:])
                    nc.scalar.activation(e_sb[:, blk, :], sc[:], AFT.Exp)

            def emit_pv(qh):
                # per j-slice: one [128,512] attn accumulator (both heads)
                # + one [2,512] denominator accumulator over 32 blocks
                for j in range(2):
                    o_ps = ppool.tile([P, 512], FP32, tag="ps",
                                      name=f"o_ps{qh}_{j}")
                    dn_ps = ppool.tile([2, 512], FP32, tag="ps",
                                       name=f"dn_ps{qh}_{j}")
                    for blk in range(NBLK):
                        nc.tensor.matmul(
                            o_ps[:], vbd[:, blk, :],
                            e_sb[:, blk, ts(j, 512)],
                            start=(blk == 0), stop=(blk == NBLK - 1))
                    for blk in range(NBLK):
                        nc.tensor.matmul(
                            dn_ps[:], dn_w[:], e_sb[:, blk, ts(j, 512)],
                            start=(blk == 0), stop=(blk == NBLK - 1))
                    qsl = ds(qh * 1024 + j * 512, 512)
                    rec = spool.tile([2, 512], FP32, tag="rec", name="rec")
                    nc.vector.reciprocal(rec[:], dn_ps[:])
                    recb = spool.tile([2, 512], BF16, tag="recb",
                                      name="recb")
                    nc.vector.tensor_copy(recb[:], rec[:])
                    bc = ppool.tile([P, 512], FP32, tag="ps", name="bc")
                    nc.tensor.matmul(bc[:], dn_sel[:], recb[:],
                                     start=True, stop=True)
                    bc_sb = spool.tile([P, 512], FP32, tag="bcsb",
                                       name="bcsb")
                    nc.vector.tensor_copy(bc_sb[:], bc[:])
                    nc.vector.tensor_mul(attn_sb[:, qsl], o_ps[:], bc_sb[:])
                nc.sync.dma_start(ag_in[qh][:],
                                  attn_sb[:, ds(qh * 1024, 1024)])

            def emit_pre():
                emit_xdma()
                emit_proj("k", range(NB))
                emit_proj("q", range(NB))
                emit_proj("v", range(NB))
                emit_kbd()
                emit_vbd()
                if part in ("scores", "pre", "full"):
                    emit_scores(0)
                    if part != "scores":
                        emit_pv(0)
                    emit_scores(1)
                    if part != "scores":
                        emit_pv(1)

            def emit_post():
                for nb in range(2):
                    o_ps = [ppool.tile([P, 512], FP32, tag="ps",
                                       name=f"out_ps{nb}_{j}")
                            for j in range(2)]
                    rts = []
                    for kc in range(KC):
                        rt = rpool.tile([P, 1024], BF16, tag="rhs",
                                        name=f"rt{nb}_{kc}")
                        nc.sync.dma_start(
                            rt[:], ag_out[nb][ds(kc * P, P), :])
                        rts.append(rt)
                    for kc in range(KC):
                        for j in range(2):
                            nc.tensor.matmul(
                                o_ps[j][:], w_sb["o"][:, kc, :],
                                rts[kc][:, ts(j, 512)],
                                start=(kc == 0), stop=(kc == KC - 1))
                    for j in range(2):
                        nc.vector.tensor_scalar_add(
                            out_sb[:, ds(nb * 1024 + j * 512, 512)],
                            o_ps[j][:], bo_sb[:])
                    nc.sync.dma_start(out_d.ap()[:, ts(nb, 1024)],
                                      out_sb[:, ts(nb, 1024)])

            if loop_r is None:
                for _ in range(reps):
                    emit_pre()
                for i in range(2):
                    nc.gpsimd.collective_compute(
                        "AllGather", mybir.AluOpType.bypass,
                        ins=[ag_in[i].opt()], outs=[ag_out[i].opt()],
                        replica_groups=[list(range(N_CORES))])
                for _ in range(reps):
                    emit_post()
            else:
                for i in range(2):
                    nc.gpsimd.collective_compute(
                        "AllGather", mybir.AluOpType.bypass,
                        ins=[ag_in[i].opt()], outs=[ag_out[i].opt()],
                        replica_groups=[list(range(N_CORES))])
                with tc.For_i(0, loop_r, 1):
                    emit_pre()
                    if part == "full":
                        emit_post()
    nc.finalize()
    return nc


_NC = None


def _get_nc():
    global _NC
    if _NC is None:
        _NC = build()
    return _NC


def _dnw():
    w = np.zeros((P, 2), ml_dtypes.bfloat16)
    w[0:D, 0] = 1
    w[D:P, 1] = 1
    return w


def _dnsel():
    s = np.zeros((2, P), ml_dtypes.bfloat16)
    s[0, 0:D] = 1
    s[1, D:P] = 1
    return s


def make_in_maps(hidden_states, Wq, Wk, Wv, Wo, bo):
    x = np.ascontiguousarray(
        np.asarray(hidden_states, np.float32).reshape(C, S))
    scale = np.float32(D ** -0.5)
    Wq = np.asarray(Wq, np.float32)
    Wk = np.asarray(Wk, np.float32)
    Wv = np.asarray(Wv, np.float32)
    Wo = np.asarray(Wo, np.float32)
    bo = np.asarray(bo, np.float32)
    in_maps = []
    for i in range(N_CORES):
        sl = slice(i * P, (i + 1) * P)
        in_maps.append({
            "x": x.astype(ml_dtypes.bfloat16),
            "wqT": np.ascontiguousarray((Wq[sl] * scale).T).astype(ml_dtypes.bfloat16),
            "wkT": np.ascontiguousarray(Wk[sl].T).astype(ml_dtypes.bfloat16),
            "wvT": np.ascontiguousarray(Wv[sl].T).astype(ml_dtypes.bfloat16),
            "woT": np.ascontiguousarray(Wo[sl].T).astype(ml_dtypes.bfloat16),
            "bo": np.ascontiguousarray(bo[sl].reshape(P, 1)),
            "ident": np.eye(P, dtype=ml_dtypes.bfloat16),
            "ones": np.ones((P, 1), ml_dtypes.bfloat16),
            "dnw": _dnw(),
            "dnsel": _dnsel(),
        })
    return in_maps


def kernel(hidden_states, Wq, Wk, Wv, Wo, bo):
    nc = _get_nc()
    in_maps = make_in_maps(hidden_states, Wq, Wk, Wv, Wo, bo)
    res = run_bass_kernel_spmd(nc, in_maps, core_ids=list(range(N_CORES)))
    out = np.concatenate([res.results[i]["out"] for i in range(N_CORES)],
                         axis=0)
    return out.reshape(1, C, 1, S)


# revision 31
# speedup vs baseline: 1.0641x; 1.0641x over previous
"""Distributed multi-head attention kernel for 8 TRN2 NeuronCores (v4).

Reference problem (hardcoded):
    hidden_states [1, 1024, 1, 2048] f32, Wq/Wk/Wv [1024, 1024],
    Wo [1024, 1024], bo [1024].  16 heads x 64 dim, seq 2048.

Sharding: tensor-parallel over heads.  Core i computes heads (2i, 2i+1):
QKV projections for its 128 channels, per-head scores, exp, PV, normalize,
AllGather of the attn block, then a row shard of the output projection
(+bias).  Host concatenates row shards.

v4 — all matmuls K=128 via BLOCK-DIAGONAL head packing (HW calibration
showed K=64 matmuls run at ~2x cost/col and single-shot K=64 at ~443 ns vs
~246 ns for K=128):
  - scores: stationary kbd[.., blk, ..] is [128, 128] block-diagonal
    (k_h0 in the top-left 64x64, k_h1 bottom-right); moving q_sb [128, n]
    holds both heads' d stacked.  Output tile row p<64 = S_h0[kpos, q],
    p>=64 = S_h1[kpos, q] for the same 64 k-positions `blk`.
  - exp: DVE copies score tiles PSUM->SBUF bf16 (~0.5 ns/col; the v2
    kernel's ACT-exp-from-PSUM ran at ~5.7 ns/col and dominated), ACT
    exps bf16->bf16 from SBUF (~0.2 ns/col).
  - PV: stationary vbd[.., blk, ..] is [128, 128] block-diagonal vT built
    from two partition-offset transposes per 64-block; output rows are the
    128 attn channels directly.
  - softmax denominators: ones-pair stationary [128, 2] summing rows 0:64
    (h0) and 64:128 (h1) of each exp tile into a [2, 512] accumulator.
  - x DMA in [128, 1024] chunks (2 KB lines; 1 KB lines ran at ~half BW).
  - post prefetches the gathered-attn row blocks 4 deep.
bench mode (loop_r=N) wraps pre+post in a single hardware For_i.
"""

import numpy as np
import ml_dtypes

import concourse.bass as bass
import concourse.mybir as mybir
import concourse.tile as tile
from concourse import bacc
from concourse.bass import ts, ds
from concourse.bass_utils import run_bass_kernel_spmd

S = 2048          # sequence length
C = 1024          # query dim == inner dim
P = 128           # partitions / per-core channel count
D = 64            # head dim
HC = 2            # heads per core
N_CORES = 8
KC = C // P       # 8 contraction chunks for the projections
NBLK = S // D     # 32 key-position blocks of 64
NB = S // 512     # 4 free-dim blocks of 512
FP32 = mybir.dt.float32
BF16 = mybir.dt.bfloat16
AFT = mybir.ActivationFunctionType


def build(loop_r=None, reps=1, part="full"):
    nc = bacc.Bacc("TRN2", target_bir_lowering=False, debug=False,
                   num_devices=N_CORES)
    x_d = nc.dram_tensor("x", [C, S], BF16, kind="ExternalInput")
    wq_d = nc.dram_tensor("wqT", [C, P], BF16, kind="ExternalInput")
    wk_d = nc.dram_tensor("wkT", [C, P], BF16, kind="ExternalInput")
    wv_d = nc.dram_tensor("wvT", [C, P], BF16, kind="ExternalInput")
    wo_d = nc.dram_tensor("woT", [C, P], BF16, kind="ExternalInput")
    bo_d = nc.dram_tensor("bo", [P, 1], FP32, kind="ExternalInput")
    id_d = nc.dram_tensor("ident", [P, P], BF16, kind="ExternalInput")
    ones_d = nc.dram_tensor("ones", [P, 1], BF16, kind="ExternalInput")
    dnw_d = nc.dram_tensor("dnw", [P, 2], BF16, kind="ExternalInput")
    dnsel_d = nc.dram_tensor("dnsel", [2, P], BF16, kind="ExternalInput")
    out_d = nc.dram_tensor("out", [P, S], FP32, kind="ExternalOutput")

    with tile.TileContext(nc) as tc:
        with (
            tc.tile_pool(name="const", bufs=1) as cpool,
            tc.tile_pool(name="big", bufs=1) as big,
            tc.tile_pool(name="sc", bufs=3) as scpool,
            tc.tile_pool(name="rhs", bufs=4) as rpool,
            tc.tile_pool(name="small", bufs=4) as spool,
            tc.tile_pool(name="psum", bufs=4, space="PSUM") as ppool,
            tc.tile_pool(name="stp", bufs=2, space="PSUM") as stpool,
            tc.tile_pool(name="dram", bufs=1, space="DRAM") as dpool,
        ):
            # ---- constants / weights (outside any bench loop) ----
            ident = cpool.tile([P, P], BF16, tag="ident")
            nc.sync.dma_start(ident[:], id_d.ap())
            ones_sb = cpool.tile([P, 1], BF16, tag="ones")
            nc.sync.dma_start(ones_sb[:], ones_d.ap())
            w_sb = {}
            for name, dram in (("q", wq_d), ("k", wk_d), ("v", wv_d),
                               ("o", wo_d)):
                t = cpool.tile([P, KC, P], BF16, tag=f"w{name}")
                nc.sync.dma_start(
                    t[:], dram.ap().rearrange("(kc p) m -> p kc m", p=P))
                w_sb[name] = t
            bo_sb = cpool.tile([P, 1], FP32, tag="bo")
            nc.sync.dma_start(bo_sb[:], bo_d.ap())
            # absorb the exp table load into the DMA lead-in
            warm = cpool.tile([P, 1], FP32, tag="warm")
            nc.scalar.activation(warm[:], bo_sb[:], AFT.Exp)
            # ones-pair stationary for the denominator matmuls:
            # col 0 sums rows 0:64 (h0), col 1 sums rows 64:128 (h1)
            dn_w = cpool.tile([P, 2], BF16, tag="dnw")
            nc.sync.dma_start(dn_w[:], dnw_d.ap())
            # selector for the PE denominator broadcast: col c takes rec row
            # 0 (c < 64) or row 1 (c >= 64)
            dn_sel = cpool.tile([2, P], BF16, tag="dnsel")
            nc.sync.dma_start(dn_sel[:], dnsel_d.ap())
            # block-diagonal stationaries; the zero halves are written once
            kbd = big.tile([P, NBLK, P], BF16, tag="kbd")
            vbd = big.tile([P, NBLK, P], BF16, tag="vbd")
            nc.vector.memset(kbd[:], 0.0)
            nc.vector.memset(vbd[:], 0.0)

            x_sb = big.tile([P, KC, S], BF16, tag="x")
            proj = {}
            for name in ("q", "v"):
                proj[name] = big.tile([P, S], BF16, tag=f"{name}sb",
                                      name=f"{name}sb")
            e_sb = big.tile([P, NBLK, S // 2], BF16, tag="esb")
            attn_sb = big.tile([P, S], BF16, tag="attn")
            out_sb = big.tile([P, S], FP32, tag="outsb")
            ag_in = [dpool.tile([P, S // 2], BF16, tag=f"agin{i}",
                                name=f"agin{i}") for i in range(2)]
            ag_out = [dpool.tile([C, S // 2], BF16, tag=f"agout{i}",
                                 addr_space="Shared", name=f"agout{i}")
                      for i in range(2)]

            def emit_xdma():
                # x into SBUF in [128, 1024] chunks (2 KB DMA lines)
                x_view = x_d.ap().rearrange("(kc p) s -> kc p s", kc=KC)
                for half in range(2):
                    for kc in range(KC):
                        nc.sync.dma_start(
                            x_sb[:, kc, ds(half * 1024, 1024)],
                            x_view[kc][:, ds(half * 1024, 1024)])

            def emit_proj(name, nbs):
                # name "k": write the block-diagonal kbd layout directly
                # (one strided copy per 64-row half covering 8 blocks)
                for nb in nbs:
                    ps = ppool.tile([P, 512], FP32, tag="ps",
                                    name=f"{name}{nb}_ps")
                    for kc in range(KC):
                        nc.tensor.matmul(
                            ps[:], w_sb[name][:, kc, :],
                            x_sb[:, kc, ts(nb, 512)],
                            start=(kc == 0), stop=(kc == KC - 1))
                    if name == "k":
                        b0 = nb * 8
                        nc.vector.tensor_copy(
                            kbd[0:D, b0:b0 + 8, 0:D],
                            ps[0:D, :].rearrange("p (b c) -> p b c", b=8))
                        nc.vector.tensor_copy(
                            kbd[D:P, b0:b0 + 8, D:P],
                            ps[D:P, :].rearrange("p (b c) -> p b c", b=8))
                    else:
                        nc.vector.tensor_copy(proj[name][:, ts(nb, 512)],
                                              ps[:])

            def emit_vbd():
                # two partition-offset transposes per 64-block, then
                # partition-aligned copies (on ACT) into the block-diagonal
                # layout
                v_sb = proj["v"]
                for blk in range(NBLK):
                    tp = ppool.tile([P, P], BF16, tag="ps", name=f"tp{blk}")
                    nc.tensor.transpose(tp[0:D, :], v_sb[:, ts(blk, D)],
                                        ident[:])
                    nc.tensor.transpose(tp[D:P, :], v_sb[:, ts(blk, D)],
                                        ident[:])
                    nc.vector.tensor_copy(vbd[0:D, blk, 0:D], tp[0:D, 0:D])
                    nc.vector.tensor_copy(vbd[D:P, blk, D:P], tp[D:P, D:P])

            def emit_scores(qh):
                # per blk: [128,128] block-diag stationary, 2 single-shot
                # matmuls into one [128,1024] st tile; one DVE copy
                # PSUM->SBUF bf16; ACT exps into e_sb
                q_sb = proj["q"]
                for blk in range(NBLK):
                    st = stpool.tile([P, 1024], FP32, tag="st",
                                     name=f"st{blk}")
                    for j in range(2):
                        nc.tensor.matmul(
                            st[:, ts(j, 512)], kbd[:, blk, :],
                            q_sb[:, ds(qh * 1024 + j * 512, 512)],
                            start=True, stop=True)
                    sc = scpool.tile([P, S // 2], BF16, tag="sc",
                                     name=f"sc{blk}")
                    nc.vector.tensor_copy(sc[:], st[:])
                    nc.scalar.activation(e_sb[:, blk, :], sc[:], AFT.Exp)

            def emit_pv(qh):
                # per j-slice: one [2,512] denominator accumulator (first,
                # so the reciprocal chain overlaps the attn accumulation)
                # + one [128,512] attn accumulator over 32 blocks; the
                # per-head reciprocal broadcast is a K=2 PE matmul
                o_ps, dn_ps, recb = [], [], []
                for j in range(2):
                    dn_ps.append(ppool.tile([2, 512], FP32, tag="ps",
                                            name=f"dn_ps{qh}_{j}"))
                    o_ps.append(ppool.tile([P, 512], FP32, tag="ps",
                                           name=f"o_ps{qh}_{j}"))
                for j in range(2):
                    for blk in range(NBLK):
                        nc.tensor.matmul(
                            dn_ps[j][:], dn_w[:], e_sb[:, blk, ts(j, 512)],
                            start=(blk == 0), stop=(blk == NBLK - 1))
                    rec = spool.tile([2, 512], FP32, tag="rec", name="rec")
                    nc.vector.reciprocal(rec[:], dn_ps[j][:])
                    rb = spool.tile([2, 512], BF16, tag="recb", name="recb")
                    nc.vector.tensor_copy(rb[:], rec[:])
                    recb.append(rb)
                    for blk in range(NBLK):
                        nc.tensor.matmul(
                            o_ps[j][:], vbd[:, blk, :],
                            e_sb[:, blk, ts(j, 512)],
                            start=(blk == 0), stop=(blk == NBLK - 1))
                for j in range(2):
                    qsl = ds(qh * 1024 + j * 512, 512)
                    bc = ppool.tile([P, 512], FP32, tag="ps", name="bc")
                    nc.tensor.matmul(bc[:], dn_sel[:], recb[j][:],
                                     start=True, stop=True)
                    bc_sb = spool.tile([P, 512], FP32, tag="bcsb",
                                       name="bcsb")
                    nc.vector.tensor_copy(bc_sb[:], bc[:])
                    nc.vector.tensor_mul(attn_sb[:, qsl], o_ps[j][:],
                                         bc_sb[:])
                nc.sync.dma_start(ag_in[qh][:],
                                  attn_sb[:, ds(qh * 1024, 1024)])

            def emit_pre():
                emit_xdma()
                emit_proj("k", range(NB))
                emit_proj("q", range(NB))
                emit_proj("v", range(NB))
                emit_vbd()
                if part in ("scores", "pre", "full"):
                    emit_scores(0)
                    if part != "scores":
                        emit_pv(0)
                    emit_scores(1)
                    if part != "scores":
                        emit_pv(1)

            def emit_post():
                for nb in range(2):
                    o_ps = [ppool.tile([P, 512], FP32, tag="ps",
                                       name=f"out_ps{nb}_{j}")
                            for j in range(2)]
                    rts = []
                    for kc in range(KC):
                        rt = rpool.tile([P, 1024], BF16, tag="rhs",
                                        name=f"rt{nb}_{kc}")
                        nc.sync.dma_start(
                            rt[:], ag_out[nb][ds(kc * P, P), :])
                        rts.append(rt)
                    for kc in range(KC):
                        for j in range(2):
                            nc.tensor.matmul(
                                o_ps[j][:], w_sb["o"][:, kc, :],
                                rts[kc][:, ts(j, 512)],
                                start=(kc == 0), stop=(kc == KC - 1))
                    for j in range(2):
                        nc.vector.tensor_scalar_add(
                            out_sb[:, ds(nb * 1024 + j * 512, 512)],
                            o_ps[j][:], bo_sb[:])
                    nc.sync.dma_start(out_d.ap()[:, ts(nb, 1024)],
                                      out_sb[:, ts(nb, 1024)])

            if loop_r is None:
                for _ in range(reps):
                    emit_pre()
                for i in range(2):
                    nc.gpsimd.collective_compute(
                        "AllGather", mybir.AluOpType.bypass,
                        ins=[ag_in[i].opt()], outs=[ag_out[i].opt()],
                        replica_groups=[list(range(N_CORES))])
                for _ in range(reps):
                    emit_post()
            else:
                for i in range(2):
                    nc.gpsimd.collective_compute(
                        "AllGather", mybir.AluOpType.bypass,
                        ins=[ag_in[i].opt()], outs=[ag_out[i].opt()],
                        replica_groups=[list(range(N_CORES))])
                with tc.For_i(0, loop_r, 1):
                    emit_pre()
                    if part == "full":
                        emit_post()
    nc.finalize()
    return nc


_NC = None


def _get_nc():
    global _NC
    if _NC is None:
        _NC = build()
    return _NC


def _dnw():
    w = np.zeros((P, 2), ml_dtypes.bfloat16)
    w[0:D, 0] = 1
    w[D:P, 1] = 1
    return w


def _dnsel():
    s = np.zeros((2, P), ml_dtypes.bfloat16)
    s[0, 0:D] = 1
    s[1, D:P] = 1
    return s


def make_in_maps(hidden_states, Wq, Wk, Wv, Wo, bo):
    x = np.ascontiguousarray(
        np.asarray(hidden_states, np.float32).reshape(C, S))
    scale = np.float32(D ** -0.5)
    Wq = np.asarray(Wq, np.float32)
    Wk = np.asarray(Wk, np.float32)
    Wv = np.asarray(Wv, np.float32)
    Wo = np.asarray(Wo, np.float32)
    bo = np.asarray(bo, np.float32)
    in_maps = []
    for i in range(N_CORES):
        sl = slice(i * P, (i + 1) * P)
        in_maps.append({
            "x": x.astype(ml_dtypes.bfloat16),
            "wqT": np.ascontiguousarray((Wq[sl] * scale).T).astype(ml_dtypes.bfloat16),
            "wkT": np.ascontiguousarray(Wk[sl].T).astype(ml_dtypes.bfloat16),
            "wvT": np.ascontiguousarray(Wv[sl].T).astype(ml_dtypes.bfloat16),
            "woT": np.ascontiguousarray(Wo[sl].T).astype(ml_dtypes.bfloat16),
            "bo": np.ascontiguousarray(bo[sl].reshape(P, 1)),
            "ident": np.eye(P, dtype=ml_dtypes.bfloat16),
            "ones": np.ones((P, 1), ml_dtypes.bfloat16),
            "dnw": _dnw(),
            "dnsel": _dnsel(),
        })
    return in_maps


def kernel(hidden_states, Wq, Wk, Wv, Wo, bo):
    nc = _get_nc()
    in_maps = make_in_maps(hidden_states, Wq, Wk, Wv, Wo, bo)
    res = run_bass_kernel_spmd(nc, in_maps, core_ids=list(range(N_CORES)))
    out = np.concatenate([res.results[i]["out"] for i in range(N_CORES)],
                         axis=0)
    return out.reshape(1, C, 1, S)
